# revision 45
# baseline (speedup 1.0000x reference)
"""AttentionCritic kernel for 8 Trainium2 NeuronCores.

Data-parallel over batch: B=8192 -> 8 cores x B_LOC=1024. No collectives.

v3: cost-model-driven rewrite of the batch-paired design.
  - encoder psum [128, 512] per agent (both H-halves), ONE merged Prelu
  - Q/K projections 2-head-packed (M=128) with agent-paired N=512 matmuls
    (halves PE streaming cycles vs per-head M=64)
  - slab quadrant copies widened to [64, 2a x 128] and split across
    DVE / ACT / GPSIMD(Pool)
  - V via e-stationary M=128 matmuls: lhsT free dims (q4, a16, bp2) over a
    16-agent padded e_sl -> [32q + 2a + bp, (kk,g,d)] tiles, 2 pair-slots
    per psum bank, one merged Prelu per bank
  - logits^T pair-matmuls as v2 (N=30, zero-quadrant slabs); mlog NEG
    memsets hoisted out of the loop (masked-off cells stay NEG forever)
  - attention pair-matmul [(bp,i), d]; P pre-normalized
  - output: per-(g,chunk) single 5-dim DMA from osb
"""

import sys

sys.path.insert(0, "/opt/trn_rl_repo")

import numpy as np
from ml_dtypes import bfloat16

import concourse.bass as bass
import concourse.bacc as bacc
from concourse import mybir
from concourse.tile import TileContext
from concourse.alu_op_type import AluOpType
from concourse.bass_utils import run_bass_kernel_spmd

A, S, ACT, H, G, D = 15, 128, 32, 256, 4, 64
AP16 = 16                     # padded agent count for M=128 V matmuls
N_CORES = 8
B_FULL = 8192
B_LOC = B_FULL // N_CORES
CB = 256                      # batch chunk per pipeline stage
HB = CB // 2                  # half-chunk = pair stride (128)
NCHUNK = B_LOC // CB
HH = 2                        # h halves (H=256 -> 2x128)
NEG = -1000.0                 # masked logit value (exp(NEG/8) == 0)

FP32 = mybir.dt.float32
BF16 = mybir.dt.bfloat16
U8 = mybir.dt.uint8

SLOTS = 16                    # col-slots per logits bank
LGF = SLOTS * 2 * A           # 480 free elems in logits bank


def _build(nc: bass.Bass, bias_enc: bool, bias_s: bool, bias_v: bool):
    dp = nc.declare_dram_parameter
    statesT_p = dp("statesT_p", [S, A * B_LOC], BF16, isOutput=False)
    actionsT_p = dp("actionsT_p", [ACT, A * B_LOC], BF16, isOutput=False)
    wenc1 = dp("wenc1", [S, A * H], BF16, isOutput=False)        # [s, (a,h)]
    wenc2 = dp("wenc2", [ACT, A * H], BF16, isOutput=False)
    ws_p = dp("ws_p", [S, A * H], BF16, isOutput=False)
    # [h, (p(q/k), gpair, hh, g', d)]
    wqk2 = dp("wqk2", [128, 2 * 2 * HH * 128], BF16, isOutput=False)
    wv_p = dp("wv_p", [128, HH * G * D], BF16, isOutput=False)   # [h,(hh,g,d)]
    benc_p = dp("benc_p", [128, HH * A], FP32, isOutput=False)
    bs_p = dp("bs_p", [128, HH * A], FP32, isOutput=False)
    bvrow = dp("bvrow", [1, G * D], BF16, isOutput=False)
    onesrow = dp("onesrow", [1, 128], BF16, isOutput=False)
    maskkeep = dp("maskkeep", [128, 2 * A], U8, isOutput=False)  # [32c+2j+bp,(2i+bp')]
    denones = dp("denones", [128, 8], BF16, isOutput=False)
    expand = dp("expand", [8, 128], FP32, isOutput=False)
    out = dp("out", [G, A, B_LOC, D], BF16, isOutput=True)

    with TileContext(nc) as tc, \
         tc.tile_pool(name="consts", bufs=1) as cpool, \
         tc.tile_pool(name="work", bufs=1) as wpool, \
         tc.tile_pool(name="psum", bufs=2, space="PSUM") as ppool:

        def cload(name, src, P, F, dt):
            t = cpool.tile([P, F], dt, tag=name, name=name)
            nc.sync.dma_start(out=t[:, :], in_=src[:, :])
            return t

        c_wenc1 = cload("c_wenc1", wenc1, S, A * H, BF16)
        c_wenc2 = cload("c_wenc2", wenc2, ACT, A * H, BF16)
        c_ws = cload("c_ws", ws_p, S, A * H, BF16)
        if bias_enc:
            c_benc = cload("c_benc", benc_p, 128, HH * A, FP32)
        if bias_s:
            c_bs = cload("c_bs", bs_p, 128, HH * A, FP32)
        if bias_v:
            c_bvrow = cload("c_bvrow", bvrow, 1, G * D, BF16)
            c_ones = cload("c_ones", onesrow, 1, 128, BF16)

        # persistent zero-quadrant Q/K slabs: [128=(bh,d64), (g, a, b256)]
        q_sl = wpool.tile([128, G * A * CB], BF16, tag="q_sl", bufs=1)
        k_sl = wpool.tile([128, G * A * CB], BF16, tag="k_sl", bufs=1)
        nc.gpsimd.memset(q_sl[:, :], 0.0)
        nc.gpsimd.memset(k_sl[:, :], 0.0)

        # encoder outputs: e_sl cols (hh, k32, q4, a16, bp2) so each
        # 128-col block [hh,k] is a contiguous M=128 V/K-proj lhsT
        # (batch b = k + 32q + 128bp; psum row m = 32q + 2a + bp)
        e_sl = wpool.tile([128, HH * AP16 * CB], BF16, tag="e_sl", bufs=1)
        s_sl = wpool.tile([128, HH * A * CB], BF16, tag="s_sl", bufs=1)
        ev5 = e_sl[:, :].rearrange("p (hh k q a bp) -> p hh k q a bp",
                                   hh=HH, k=32, q=4, a=AP16)
        # zero once: padded-agent cells (a=15) must stay 0 for V junk rows
        nc.gpsimd.memset(e_sl[:, :], 0.0)

        # mlog buffers: NEG-init once; masked-off cells are never rewritten
        MLOG_BUFS = 3
        for _mi in range(MLOG_BUFS):
            mlog_init = wpool.tile([128, LGF], BF16, tag="mlog",
                                   bufs=MLOG_BUFS, name=f"mlog_init{_mi}")
            nc.vector.memset(mlog_init[:, :], NEG)

        # agent groups (pairs + trailing singleton)
        agroups = [(a, a + 1) if a + 1 < A else (a,) for a in range(0, A, 2)]
        # qk copy engine pattern (V=vector, A=scalar); GPSIMD cannot
        # access PSUM so Pool only issues casting DMAs (see cp_dma below)
        cp_engines = [nc.vector, nc.scalar] * 6 + [nc.vector] * 4
        # osb copies all on DVE (ACT is tighter after the e-prelu split)

        stv = statesT_p[:, :].rearrange("s (a b) -> s a b", b=B_LOC)
        acv = actionsT_p[:, :].rearrange("s (a b) -> s a b", b=B_LOC)
        in_tiles = {}

        def load_inputs(c):
            # prefetch chunk c's inputs (issued ahead of output DMAs)
            b0 = c * CB
            stT = wpool.tile([S, A * CB], BF16, tag="stT", bufs=2,
                             name=f"stT{c}")
            acT = wpool.tile([ACT, A * CB], BF16, tag="acT", bufs=2,
                             name=f"acT{c}")
            nc.sync.dma_start(
                out=stT[:, :].rearrange("s (a b) -> s a b", b=CB),
                in_=stv[:, :, b0:b0 + CB])
            nc.sync.dma_start(
                out=acT[:, :].rearrange("s (a b) -> s a b", b=CB),
                in_=acv[:, :, b0:b0 + CB])
            in_tiles[c] = (stT, acT)

        load_inputs(0)
        # less-urgent consts load after the first input chunk
        c_wqk2 = cload("c_wqk2", wqk2, 128, 2 * 2 * HH * 128, BF16)
        c_wv = cload("c_wv", wv_p, 128, HH * G * D, BF16)
        c_mask = cload("c_mask", maskkeep, 128, 2 * A, U8)
        c_dones = cload("c_dones", denones, 128, 8, BF16)
        c_exp = cload("c_exp", expand, 8, 128, FP32)
        def emit_B1(c):
            # ---------- stage B1: all encoders (deep psum rotation) ------
            stT, acT = in_tiles.pop(c)
            for grp in [(a,) for a in range(A)]:
                for a in grp:
                    eps = ppool.tile([128, 512], FP32, tag="big", bufs=8)
                    sps = ppool.tile([128, 512], FP32, tag="big", bufs=8)
                    for hh in range(HH):
                        nc.tensor.matmul(
                            eps[:, hh * 256:hh * 256 + 256],
                            c_wenc1[:, a * H + hh * 128:a * H + hh * 128 + 128],
                            stT[:, a * CB:(a + 1) * CB], start=True, stop=False)
                        nc.tensor.matmul(
                            eps[:, hh * 256:hh * 256 + 256],
                            c_wenc2[:, a * H + hh * 128:a * H + hh * 128 + 128],
                            acT[:, a * CB:(a + 1) * CB], start=False, stop=True)
                        nc.tensor.matmul(
                            sps[:, hh * 256:hh * 256 + 256],
                            c_ws[:, a * H + hh * 128:a * H + hh * 128 + 128],
                            stT[:, a * CB:(a + 1) * CB], start=True, stop=True)
                    # PSUM -> SBUF with Prelu (fused copy); e goes to the
                    # (hh,k,q,a,bp) layout via rank-3 APs, one per hh
                    sv = s_sl[:, :].rearrange("p (hh a b) -> p hh a b",
                                              hh=HH, a=A)
                    sdst = sv[:, :, a, :]
                    # eps cols (hh, b) with b = 128bp + 32q + k
                    eps5 = eps[:, :].rearrange("p (hh bp q k) -> p hh k q bp",
                                               hh=HH, bp=2, q=4, k=32)
                    sps3 = sps[:, :].rearrange("p (hh b) -> p hh b", hh=HH)
                    for hh in range(HH):
                        nc.scalar.activation(
                            ev5[:, hh, :, :, a, :], eps5[:, hh],
                            mybir.ActivationFunctionType.Prelu,
                            bias=(c_benc[:, hh * A + a:hh * A + a + 1]
                                  if bias_enc else 0.0),
                            alpha=0.01)
                    if bias_s:
                        for hh in range(HH):
                            nc.scalar.activation(
                                sdst[:, hh, :], sps3[:, hh, :],
                                mybir.ActivationFunctionType.Prelu,
                                bias=c_bs[:, hh * A + a:hh * A + a + 1],
                                alpha=0.01)
                    else:
                        nc.scalar.activation(
                            sdst, sps3,
                            mybir.ActivationFunctionType.Prelu, alpha=0.01)

        cp_i = 0
        emit_B1(0)
        for c in range(NCHUNK):
            # ---------- stage A: prefetch next chunk's inputs ------------
            if c + 1 < NCHUNK:
                load_inputs(c + 1)

            # ---------- stage B2: 2-head-packed Q/K proj + slab copies ---
            qv3 = q_sl[:, :].rearrange("p (g a b) -> p g a b", g=G, a=A)
            kv3 = k_sl[:, :].rearrange("p (g a b) -> p g a b", g=G, a=A)

            def emit_copy(dst, src):
                nonlocal cp_i
                eng = cp_engines[cp_i % len(cp_engines)]
                cp_i += 1
                if eng is nc.scalar:
                    nc.scalar.copy(dst, src)
                else:
                    eng.tensor_copy(dst, src)

            # Q projection from s_sl (agent-contiguous): psum [(g',d),(a2,b)]
            for grp in agroups:
                a0, na = grp[0], len(grp)
                w = na * CB
                for gp in range(2):
                    qps = ppool.tile([128, 512], FP32, tag="big", bufs=8)
                    for hh in range(HH):
                        wq_s = c_wqk2[:, ((0 * 2 + gp) * 2 + hh) * 128:
                                      ((0 * 2 + gp) * 2 + hh) * 128 + 128]
                        nc.tensor.matmul(
                            qps[:, :w],
                            wq_s,
                            s_sl[:, hh * A * CB + a0 * CB:
                                 hh * A * CB + a0 * CB + w],
                            start=(hh == 0), stop=(hh == 1))
                    p3 = qps[:, :w].rearrange("p (a b) -> p a b", a=na)
                    for gq in range(2):
                        g = 2 * gp + gq
                        for bh in range(2):
                            emit_copy(
                                qv3[64 * bh:64 * bh + 64, g, a0:a0 + na,
                                    HB * bh:HB * bh + HB],
                                p3[64 * gq:64 * gq + 64, :,
                                   HB * bh:HB * bh + HB])

            # K projection from e_sl (k-block-contiguous): psum cols
            # (kk4, q4, a16, bp2); copies walk (a, q, kk)
            for gp in range(2):
                for kg in range(8):
                    kps = ppool.tile([128, 512], FP32, tag="big", bufs=8)
                    for kk in range(4):
                        k = 4 * kg + kk
                        for hh in range(HH):
                            wk_s = c_wqk2[:, ((1 * 2 + gp) * 2 + hh) * 128:
                                          ((1 * 2 + gp) * 2 + hh) * 128 + 128]
                            nc.tensor.matmul(
                                kps[:, 128 * kk:128 * kk + 128],
                                wk_s,
                                e_sl[:, hh * 4096 + 128 * k:
                                     hh * 4096 + 128 * k + 128],
                                start=(hh == 0), stop=(hh == 1))
                    k5 = kps[:, :].rearrange("p (kk q a bp) -> p kk q a bp",
                                             kk=4, q=4, a=AP16)
                    for gq in range(2):
                        g = 2 * gp + gq
                        for bh in range(2):
                            src = k5[64 * gq:64 * gq + 64, :, :, :A, bh]
                            src = src.rearrange("p kk q a -> p a q kk")
                            dstv = kv3[64 * bh:64 * bh + 64, g, :, :]
                            dstv = dstv.rearrange(
                                "p a (b2 q w) -> p b2 a q w", b2=2, q=4)
                            dst = dstv[:, bh, :, :, 4 * kg:4 * kg + 4]
                            emit_copy(dst, src)

            # ---------- stage C: V tiles [(32q+2a+bp), (kk,g,d)] ---------
            # lhsT = one contiguous 128-col e block -> M=128 in one matmul
            v_tiles = []
            for t in range(16):
                vps = ppool.tile([128, 512], FP32, tag="big", bufs=8)
                for kk in range(2):
                    k = 2 * t + kk
                    for hh in range(HH):
                        nc.tensor.matmul(
                            vps[:, kk * 256:kk * 256 + 256],
                            e_sl[:, hh * 4096 + 128 * k:
                                 hh * 4096 + 128 * k + 128],
                            c_wv[:, hh * G * D:(hh + 1) * G * D],
                            start=(hh == 0), stop=(hh == 1 and not bias_v))
                    if bias_v:
                        nc.tensor.matmul(
                            vps[:, kk * 256:kk * 256 + 256],
                            c_ones[:1, :], c_bvrow[:1, :],
                            start=False, stop=True)
                vsb = wpool.tile([128, 512], BF16, tag="vsb", bufs=18)
                nc.scalar.activation(
                    vsb[:, :], vps[:, :], mybir.ActivationFunctionType.Prelu,
                    alpha=0.01)
                v_tiles.append(vsb)

            # ---------- stage D1: paired logits^T + mask + exp -----------
            # beta (g, j): pairs bc = 32q + 16j + r, slot s=r, rowblock c=q
            kv4 = k_sl[:, :].rearrange("p (g a b) -> p g a b", g=G, a=A)
            qv4 = q_sl[:, :].rearrange("p (g a b) -> p g a b", g=G, a=A)
            mk = c_mask[:, :].unsqueeze(1).broadcast_to((128, SLOTS, 2 * A))
            e_tiles = []
            for beta in range(2 * G):
                g, j = beta // 2, beta % 2
                lgb = ppool.tile([128, 512], FP32, tag="big", bufs=8)
                lg = lgb[:, :LGF]
                for q in range(4):
                    for r in range(SLOTS):
                        bc = 32 * q + 16 * j + r
                        lhsT = kv4[:, g, :, bc:bc + HB + 1:HB]
                        rhs = qv4[:, g, :, bc:bc + HB + 1:HB]
                        nc.tensor.matmul(
                            lg[32 * q:32 * q + 2 * A,
                               2 * A * r:2 * A * r + 2 * A],
                            lhsT, rhs, start=True, stop=True,
                            tile_position=(0, 32 * q))
                mlog = wpool.tile([128, LGF], BF16, tag="mlog",
                                  bufs=MLOG_BUFS)
                lg3 = lg.rearrange("p (s i) -> p s i", i=2 * A)
                ml3 = mlog[:, :].rearrange("p (s i) -> p s i", i=2 * A)
                nc.vector.copy_predicated(ml3, mk, lg3)
                expm = wpool.tile([128, LGF], BF16, tag="expm", bufs=9)
                nc.scalar.activation(
                    expm[:, :], mlog[:, :], mybir.ActivationFunctionType.Exp,
                    scale=0.125)
                e_tiles.append(expm)

            # ---------- cross-chunk pipeline: next chunk's encoders ------
            # e_sl/s_sl readers of chunk c (B2 projections, V) are done;
            # B1(c+1) matmuls fill PE while ACT/DVE run softmax(c)
            if c + 1 < NCHUNK:
                emit_B1(c + 1)

            # ---------- stage D2: denominators + P = expm/den ------------
            # software-pipelined: den(b+1) covers the recip(b) latency
            rden_tiles = [None] * (2 * G)
            p_tiles = [None] * (2 * G)

            def emit_den(beta):
                den = ppool.tile([128, 512], FP32, tag="big", bufs=8)
                nc.tensor.matmul(den[:8, :LGF], c_dones[:, :],
                                 e_tiles[beta][:, :], start=True, stop=True)
                dsafe = wpool.tile([8, LGF], FP32, tag="dsafe", bufs=3)
                nc.vector.tensor_scalar_max(dsafe[:, :], den[:8, :LGF], 1e-6)
                rden = wpool.tile([8, LGF], FP32, tag="rden", bufs=3)
                nc.vector.reciprocal_approx_fast(rden[:, :], dsafe[:, :])
                rden_tiles[beta] = rden

            def emit_rrep(beta):
                rrep = ppool.tile([128, 512], FP32, tag="big", bufs=8)
                nc.tensor.matmul(rrep[:, :LGF], c_exp[:, :],
                                 rden_tiles[beta][:, :], start=True, stop=True)
                ptile = wpool.tile([128, LGF], BF16, tag="ptile", bufs=9)
                nc.vector.tensor_tensor(ptile[:, :], e_tiles[beta][:, :],
                                        rrep[:, :LGF], AluOpType.mult)
                p_tiles[beta] = ptile

            emit_den(0)
            for beta in range(1, 2 * G):
                emit_den(beta)
                emit_rrep(beta - 1)
            emit_rrep(2 * G - 1)

            # ---------- stage E: paired attention + output ----------
            for g in range(4):
                osb = wpool.tile([128, 4 * 8 * D], BF16, tag="osb", bufs=2)
                for m in range(4):
                    at = ppool.tile([128, 512], FP32, tag="big", bufs=8)
                    for q in range(4):
                        for sl in range(8):
                            bc = 32 * q + 8 * m + sl
                            s = 8 * (m % 2) + sl
                            t, kk = (8 * m + sl) // 2, sl % 2
                            pt = p_tiles[g * 2 + m // 2]
                            lhsT = pt[32 * q:32 * q + 2 * A,
                                      2 * A * s:2 * A * s + 2 * A]
                            rhs = v_tiles[t][32 * q:32 * q + 2 * A,
                                             kk * 256 + g * D:
                                             kk * 256 + (g + 1) * D]
                            nc.tensor.matmul(
                                at[32 * q:32 * q + 2 * A,
                                   sl * D:(sl + 1) * D],
                                lhsT, rhs, start=True, stop=True,
                                tile_position=(32 * q, 32 * q))
                    dst = osb[:, m * 8 * D:(m + 1) * 8 * D]
                    nc.vector.tensor_copy(dst, at[:, :])
                # b = 256c + 128bp + 32q + 8m + sl; osb row = 32q + 2a + bp
                ov = out[g, :, :, :].rearrange(
                    "a (c2 bp q m sl) d -> c2 q a bp m (sl d)",
                    c2=NCHUNK, bp=2, q=4, m=4)
                for q in range(4):
                    for bp in range(2):
                        nc.sync.dma_start(
                            out=ov[c, q, :, bp],
                            in_=osb[32 * q + bp:32 * q + bp + 2 * A:2, :]
                            .rearrange("a (m f) -> a m f", m=4))
    return nc


# ----------------------------------------------------------------------------
# host wrapper
# ----------------------------------------------------------------------------
_CACHE = {}


def _prep_consts(W_enc, b_enc, W_s, b_s, W_k, W_q, W_v, b_v):
    bf = lambda x: np.ascontiguousarray(x.astype(bfloat16))
    f32 = lambda x: np.ascontiguousarray(x.astype(np.float32))
    we = np.transpose(W_enc, (1, 0, 2))              # [160, A, H]
    wenc1 = bf(we[:S].reshape(S, A * H))
    wenc2 = bf(we[S:].reshape(ACT, A * H))
    ws_p = bf(np.transpose(W_s, (1, 0, 2)).reshape(S, A * H))

    # [h, (p, gpair, hh, g', d)]: W[p][2gp+g'][hh*128+h, d]
    wqk = np.zeros((128, 2, 2, HH, 2, D), np.float32)
    for p, W in enumerate((W_q, W_k)):
        for gp in range(2):
            for hh in range(HH):
                for gq in range(2):
                    wqk[:, p, gp, hh, gq, :] = W[2 * gp + gq,
                                                 hh * 128:(hh + 1) * 128, :]
    wqk2 = bf(wqk.reshape(128, 2 * 2 * HH * 128))

    wv = np.zeros((128, HH, G * D), np.float32)
    for hh in range(HH):
        wv[:, hh, :] = np.transpose(
            W_v[:, hh * 128:(hh + 1) * 128, :], (1, 0, 2)).reshape(128, G * D)
    wv_p = bf(wv.reshape(128, HH * G * D))
    benc_p = f32(np.transpose(b_enc.reshape(A, HH, 128), (2, 1, 0)).reshape(128, HH * A))
    bs_p = f32(np.transpose(b_s.reshape(A, HH, 128), (2, 1, 0)).reshape(128, HH * A))
    bvrow = bf(b_v.reshape(1, G * D))
    onesrow = bf(np.ones((1, 128), np.float32))
    mk = np.zeros((128, 2 * A), np.uint8)
    for cs in range(4):
        for bp in range(2):
            for jj in range(A):
                for i in range(A):
                    if jj != i:
                        mk[32 * cs + 2 * jj + bp, 2 * i + bp] = 1
    maskkeep = np.ascontiguousarray(mk)
    dn = np.zeros((128, 8), np.float32)
    for cs in range(4):
        for bp in range(2):
            for jj in range(A):
                dn[32 * cs + 2 * jj + bp, 2 * cs + bp] = 1.0
    denones = bf(dn)
    expand = f32(dn.T.copy())                        # [8, 128]
    return dict(wenc1=wenc1, wenc2=wenc2, ws_p=ws_p, wqk2=wqk2,
                wv_p=wv_p, benc_p=benc_p, bs_p=bs_p, bvrow=bvrow,
                onesrow=onesrow, maskkeep=maskkeep, denones=denones,
                expand=expand)


def kernel(states, actions, W_enc, b_enc, W_s, b_s, W_k, W_q, W_v, b_v):
    states = np.asarray(states, np.float32)
    actions = np.asarray(actions, np.float32)
    consts = _prep_consts(np.asarray(W_enc, np.float32), np.asarray(b_enc, np.float32),
                          np.asarray(W_s, np.float32), np.asarray(b_s, np.float32),
                          np.asarray(W_k, np.float32), np.asarray(W_q, np.float32),
                          np.asarray(W_v, np.float32), np.asarray(b_v, np.float32))
    key = (bool(np.any(b_enc)), bool(np.any(b_s)), bool(np.any(b_v)))
    if key not in _CACHE:
        nc = bacc.Bacc("TRN2", target_bir_lowering=False, debug=False,
                       num_devices=N_CORES)
        _build(nc, *key)
        nc.compile()
        _CACHE[key] = nc
    nc = _CACHE[key]

    in_maps = []
    for i in range(N_CORES):
        sl = slice(i * B_LOC, (i + 1) * B_LOC)
        stT = np.ascontiguousarray(
            np.transpose(states[:, sl, :], (2, 0, 1)).reshape(S, A * B_LOC).astype(bfloat16))
        acT = np.ascontiguousarray(
            np.transpose(actions[:, sl, :], (2, 0, 1)).reshape(ACT, A * B_LOC).astype(bfloat16))
        m = dict(statesT_p=stT, actionsT_p=acT)
        m.update(consts)
        in_maps.append(m)

    global _last_in_maps
    _last_in_maps = in_maps
    res = run_bass_kernel_spmd(nc, in_maps, core_ids=list(range(N_CORES)))
    outs = [np.asarray(res.results[i]["out"], dtype=np.float32)
            for i in range(N_CORES)]
    return np.concatenate(outs, axis=2)


if __name__ == "__main__":
    nc = bacc.Bacc("TRN2", target_bir_lowering=False, debug=False,
                   num_devices=N_CORES)
    _build(nc, False, False, False)
    nc.compile()
    print("build ok")


# revision 48
# speedup vs baseline: 1.0128x; 1.0128x over previous
"""AttentionCritic kernel for 8 Trainium2 NeuronCores.

Data-parallel over batch: B=8192 -> 8 cores x B_LOC=1024. No collectives.

v3: cost-model-driven rewrite of the batch-paired design.
  - encoder psum [128, 512] per agent (both H-halves), ONE merged Prelu
  - Q/K projections 2-head-packed (M=128) with agent-paired N=512 matmuls
    (halves PE streaming cycles vs per-head M=64)
  - slab quadrant copies widened to [64, 2a x 128] and split across
    DVE / ACT / GPSIMD(Pool)
  - V via e-stationary M=128 matmuls: lhsT free dims (q4, a16, bp2) over a
    16-agent padded e_sl -> [32q + 2a + bp, (kk,g,d)] tiles, 2 pair-slots
    per psum bank, one merged Prelu per bank
  - logits^T pair-matmuls as v2 (N=30, zero-quadrant slabs); mlog NEG
    memsets hoisted out of the loop (masked-off cells stay NEG forever)
  - attention pair-matmul [(bp,i), d]; P pre-normalized
  - output: per-(g,chunk) single 5-dim DMA from osb
"""

import sys

sys.path.insert(0, "/opt/trn_rl_repo")

import numpy as np
from ml_dtypes import bfloat16

import concourse.bass as bass
import concourse.bacc as bacc
from concourse import mybir
from concourse.tile import TileContext
from concourse.alu_op_type import AluOpType
from concourse.bass_utils import run_bass_kernel_spmd

A, S, ACT, H, G, D = 15, 128, 32, 256, 4, 64
AP16 = 16                     # padded agent count for M=128 V matmuls
N_CORES = 8
B_FULL = 8192
B_LOC = B_FULL // N_CORES
CB = 256                      # batch chunk per pipeline stage
HB = CB // 2                  # half-chunk = pair stride (128)
NCHUNK = B_LOC // CB
HH = 2                        # h halves (H=256 -> 2x128)
NEG = -1000.0                 # masked logit value (exp(NEG/8) == 0)

FP32 = mybir.dt.float32
BF16 = mybir.dt.bfloat16
U8 = mybir.dt.uint8

SLOTS = 16                    # col-slots per logits bank
LGF = SLOTS * 2 * A           # 480 free elems in logits bank


def _build(nc: bass.Bass, bias_enc: bool, bias_s: bool, bias_v: bool):
    dp = nc.declare_dram_parameter
    statesT_p = dp("statesT_p", [S, A * B_LOC], BF16, isOutput=False)
    actionsT_p = dp("actionsT_p", [ACT, A * B_LOC], BF16, isOutput=False)
    wenc1 = dp("wenc1", [S, A * H], BF16, isOutput=False)        # [s, (a,h)]
    wenc2 = dp("wenc2", [ACT, A * H], BF16, isOutput=False)
    ws_p = dp("ws_p", [S, A * H], BF16, isOutput=False)
    # [h, (p(q/k), gpair, hh, g', d)]
    wqk2 = dp("wqk2", [128, 2 * 2 * HH * 128], BF16, isOutput=False)
    wv_p = dp("wv_p", [128, HH * G * D], BF16, isOutput=False)   # [h,(hh,g,d)]
    benc_p = dp("benc_p", [128, HH * A], FP32, isOutput=False)
    bs_p = dp("bs_p", [128, HH * A], FP32, isOutput=False)
    bvrow = dp("bvrow", [1, G * D], BF16, isOutput=False)
    onesrow = dp("onesrow", [1, 128], BF16, isOutput=False)
    maskkeep = dp("maskkeep", [128, 2 * A], U8, isOutput=False)  # [32c+2j+bp,(2i+bp')]
    denones = dp("denones", [128, 8], BF16, isOutput=False)
    expand = dp("expand", [8, 128], FP32, isOutput=False)
    out = dp("out", [G, A, B_LOC, D], BF16, isOutput=True)

    with TileContext(nc) as tc, \
         tc.tile_pool(name="consts", bufs=1) as cpool, \
         tc.tile_pool(name="work", bufs=1) as wpool, \
         tc.tile_pool(name="psum", bufs=2, space="PSUM") as ppool:

        def cload(name, src, P, F, dt):
            t = cpool.tile([P, F], dt, tag=name, name=name)
            nc.sync.dma_start(out=t[:, :], in_=src[:, :])
            return t

        c_wenc1 = cload("c_wenc1", wenc1, S, A * H, BF16)
        c_wenc2 = cload("c_wenc2", wenc2, ACT, A * H, BF16)
        c_ws = cload("c_ws", ws_p, S, A * H, BF16)
        if bias_enc:
            c_benc = cload("c_benc", benc_p, 128, HH * A, FP32)
        if bias_s:
            c_bs = cload("c_bs", bs_p, 128, HH * A, FP32)
        if bias_v:
            c_bvrow = cload("c_bvrow", bvrow, 1, G * D, BF16)
            c_ones = cload("c_ones", onesrow, 1, 128, BF16)

        # persistent zero-quadrant Q/K slabs: [128=(bh,d64), (g, a, b256)]
        q_sl = wpool.tile([128, G * A * CB], BF16, tag="q_sl", bufs=1)
        k_sl = wpool.tile([128, G * A * CB], BF16, tag="k_sl", bufs=1)
        nc.gpsimd.memset(q_sl[:, :], 0.0)
        nc.gpsimd.memset(k_sl[:, :], 0.0)

        # encoder outputs: e_sl cols (hh, k32, q4, a16, bp2) so each
        # 128-col block [hh,k] is a contiguous M=128 V/K-proj lhsT
        # (batch b = k + 32q + 128bp; psum row m = 32q + 2a + bp)
        e_sl = wpool.tile([128, HH * AP16 * CB], BF16, tag="e_sl", bufs=1)
        s_sl = wpool.tile([128, HH * A * CB], BF16, tag="s_sl", bufs=1)
        ev5 = e_sl[:, :].rearrange("p (hh k q a bp) -> p hh k q a bp",
                                   hh=HH, k=32, q=4, a=AP16)
        # zero once: padded-agent cells (a=15) must stay 0 for V junk rows
        nc.gpsimd.memset(e_sl[:, :], 0.0)

        # mlog buffers: NEG-init once; masked-off cells are never rewritten
        MLOG_BUFS = 3
        for _mi in range(MLOG_BUFS):
            mlog_init = wpool.tile([128, LGF], BF16, tag="mlog",
                                   bufs=MLOG_BUFS, name=f"mlog_init{_mi}")
            nc.vector.memset(mlog_init[:, :], NEG)

        # agent groups (pairs + trailing singleton)
        agroups = [(a, a + 1) if a + 1 < A else (a,) for a in range(0, A, 2)]
        # qk copy engine pattern (V=vector, A=scalar); GPSIMD cannot
        # access PSUM so Pool only issues casting DMAs (see cp_dma below)
        cp_engines = [nc.vector, nc.scalar] * 7 + [nc.vector, nc.vector]
        # osb copies all on DVE (ACT is tighter after the e-prelu split)

        stv = statesT_p[:, :].rearrange("s (a b) -> s a b", b=B_LOC)
        acv = actionsT_p[:, :].rearrange("s (a b) -> s a b", b=B_LOC)
        in_tiles = {}

        def load_inputs(c):
            # prefetch chunk c's inputs (issued ahead of output DMAs)
            b0 = c * CB
            stT = wpool.tile([S, A * CB], BF16, tag="stT", bufs=2,
                             name=f"stT{c}")
            acT = wpool.tile([ACT, A * CB], BF16, tag="acT", bufs=2,
                             name=f"acT{c}")
            nc.sync.dma_start(
                out=stT[:, :].rearrange("s (a b) -> s a b", b=CB),
                in_=stv[:, :, b0:b0 + CB])
            nc.sync.dma_start(
                out=acT[:, :].rearrange("s (a b) -> s a b", b=CB),
                in_=acv[:, :, b0:b0 + CB])
            in_tiles[c] = (stT, acT)

        load_inputs(0)
        # less-urgent consts load after the first input chunk
        c_wqk2 = cload("c_wqk2", wqk2, 128, 2 * 2 * HH * 128, BF16)
        c_wv = cload("c_wv", wv_p, 128, HH * G * D, BF16)
        c_mask = cload("c_mask", maskkeep, 128, 2 * A, U8)
        c_dones = cload("c_dones", denones, 128, 8, BF16)
        c_exp = cload("c_exp", expand, 8, 128, FP32)
        def emit_B1(c):
            # ---------- stage B1: all encoders (deep psum rotation) ------
            stT, acT = in_tiles.pop(c)
            for grp in [(a,) for a in range(A)]:
                for a in grp:
                    eps = ppool.tile([128, 512], FP32, tag="big", bufs=8)
                    sps = ppool.tile([128, 512], FP32, tag="big", bufs=8)
                    for hh in range(HH):
                        nc.tensor.matmul(
                            eps[:, hh * 256:hh * 256 + 256],
                            c_wenc1[:, a * H + hh * 128:a * H + hh * 128 + 128],
                            stT[:, a * CB:(a + 1) * CB], start=True, stop=False)
                        nc.tensor.matmul(
                            eps[:, hh * 256:hh * 256 + 256],
                            c_wenc2[:, a * H + hh * 128:a * H + hh * 128 + 128],
                            acT[:, a * CB:(a + 1) * CB], start=False, stop=True)
                        nc.tensor.matmul(
                            sps[:, hh * 256:hh * 256 + 256],
                            c_ws[:, a * H + hh * 128:a * H + hh * 128 + 128],
                            stT[:, a * CB:(a + 1) * CB], start=True, stop=True)
                    # PSUM -> SBUF with Prelu (fused copy); e goes to the
                    # (hh,k,q,a,bp) layout via rank-3 APs, one per hh
                    sv = s_sl[:, :].rearrange("p (hh a b) -> p hh a b",
                                              hh=HH, a=A)
                    sdst = sv[:, :, a, :]
                    # eps cols (hh, b) with b = 128bp + 32q + k
                    eps5 = eps[:, :].rearrange("p (hh bp q k) -> p hh k q bp",
                                               hh=HH, bp=2, q=4, k=32)
                    sps3 = sps[:, :].rearrange("p (hh b) -> p hh b", hh=HH)
                    for hh in range(HH):
                        nc.scalar.activation(
                            ev5[:, hh, :, :, a, :], eps5[:, hh],
                            mybir.ActivationFunctionType.Prelu,
                            bias=(c_benc[:, hh * A + a:hh * A + a + 1]
                                  if bias_enc else 0.0),
                            alpha=0.01)
                    if bias_s:
                        for hh in range(HH):
                            nc.scalar.activation(
                                sdst[:, hh, :], sps3[:, hh, :],
                                mybir.ActivationFunctionType.Prelu,
                                bias=c_bs[:, hh * A + a:hh * A + a + 1],
                                alpha=0.01)
                    else:
                        nc.scalar.activation(
                            sdst, sps3,
                            mybir.ActivationFunctionType.Prelu, alpha=0.01)

        cp_i = 0
        emit_B1(0)
        for c in range(NCHUNK):
            # ---------- stage A: prefetch next chunk's inputs ------------
            if c + 1 < NCHUNK:
                load_inputs(c + 1)
            if c > 0:
                emit_B1(c)

            # ---------- stage B2: 2-head-packed Q/K proj + slab copies ---
            qv3 = q_sl[:, :].rearrange("p (g a b) -> p g a b", g=G, a=A)
            kv3 = k_sl[:, :].rearrange("p (g a b) -> p g a b", g=G, a=A)

            def emit_copy(dst, src):
                nonlocal cp_i
                eng = cp_engines[cp_i % len(cp_engines)]
                cp_i += 1
                if eng is nc.scalar:
                    nc.scalar.copy(dst, src)
                else:
                    eng.tensor_copy(dst, src)

            # Q projection from s_sl (agent-contiguous): psum [(g',d),(a2,b)]
            for grp in agroups:
                a0, na = grp[0], len(grp)
                w = na * CB
                for gp in range(2):
                    qps = ppool.tile([128, 512], FP32, tag="big", bufs=8)
                    for hh in range(HH):
                        wq_s = c_wqk2[:, ((0 * 2 + gp) * 2 + hh) * 128:
                                      ((0 * 2 + gp) * 2 + hh) * 128 + 128]
                        nc.tensor.matmul(
                            qps[:, :w],
                            wq_s,
                            s_sl[:, hh * A * CB + a0 * CB:
                                 hh * A * CB + a0 * CB + w],
                            start=(hh == 0), stop=(hh == 1))
                    p3 = qps[:, :w].rearrange("p (a b) -> p a b", a=na)
                    for gq in range(2):
                        g = 2 * gp + gq
                        for bh in range(2):
                            emit_copy(
                                qv3[64 * bh:64 * bh + 64, g, a0:a0 + na,
                                    HB * bh:HB * bh + HB],
                                p3[64 * gq:64 * gq + 64, :,
                                   HB * bh:HB * bh + HB])

            # K projection from e_sl (k-block-contiguous): psum cols
            # (kk4, q4, a16, bp2); copies walk (a, q, kk)
            for gp in range(2):
                for kg in range(8):
                    kps = ppool.tile([128, 512], FP32, tag="big", bufs=8)
                    for kk in range(4):
                        k = 4 * kg + kk
                        for hh in range(HH):
                            wk_s = c_wqk2[:, ((1 * 2 + gp) * 2 + hh) * 128:
                                          ((1 * 2 + gp) * 2 + hh) * 128 + 128]
                            nc.tensor.matmul(
                                kps[:, 128 * kk:128 * kk + 128],
                                wk_s,
                                e_sl[:, hh * 4096 + 128 * k:
                                     hh * 4096 + 128 * k + 128],
                                start=(hh == 0), stop=(hh == 1))
                    k5 = kps[:, :].rearrange("p (kk q a bp) -> p kk q a bp",
                                             kk=4, q=4, a=AP16)
                    for gq in range(2):
                        g = 2 * gp + gq
                        for bh in range(2):
                            src = k5[64 * gq:64 * gq + 64, :, :, :A, bh]
                            src = src.rearrange("p kk q a -> p a q kk")
                            dstv = kv3[64 * bh:64 * bh + 64, g, :, :]
                            dstv = dstv.rearrange(
                                "p a (b2 q w) -> p b2 a q w", b2=2, q=4)
                            dst = dstv[:, bh, :, :, 4 * kg:4 * kg + 4]
                            emit_copy(dst, src)

            # ---------- stage C: V tiles [(32q+2a+bp), (kk,g,d)] ---------
            # lhsT = one contiguous 128-col e block -> M=128 in one matmul
            v_tiles = []
            for t in range(16):
                vps = ppool.tile([128, 512], FP32, tag="big", bufs=8)
                for kk in range(2):
                    k = 2 * t + kk
                    for hh in range(HH):
                        nc.tensor.matmul(
                            vps[:, kk * 256:kk * 256 + 256],
                            e_sl[:, hh * 4096 + 128 * k:
                                 hh * 4096 + 128 * k + 128],
                            c_wv[:, hh * G * D:(hh + 1) * G * D],
                            start=(hh == 0), stop=(hh == 1 and not bias_v))
                    if bias_v:
                        nc.tensor.matmul(
                            vps[:, kk * 256:kk * 256 + 256],
                            c_ones[:1, :], c_bvrow[:1, :],
                            start=False, stop=True)
                vsb = wpool.tile([128, 512], BF16, tag="vsb", bufs=18)
                nc.scalar.activation(
                    vsb[:, :], vps[:, :], mybir.ActivationFunctionType.Prelu,
                    alpha=0.01)
                v_tiles.append(vsb)

            # ---------- stage D1: paired logits^T + mask + exp -----------
            # beta (g, j): pairs bc = 32q + 16j + r, slot s=r, rowblock c=q
            kv4 = k_sl[:, :].rearrange("p (g a b) -> p g a b", g=G, a=A)
            qv4 = q_sl[:, :].rearrange("p (g a b) -> p g a b", g=G, a=A)
            mk = c_mask[:, :].unsqueeze(1).broadcast_to((128, SLOTS, 2 * A))
            e_tiles = []
            for beta in range(2 * G):
                g, j = beta // 2, beta % 2
                lgb = ppool.tile([128, 512], FP32, tag="big", bufs=8)
                lg = lgb[:, :LGF]
                for q in range(4):
                    for r in range(SLOTS):
                        bc = 32 * q + 16 * j + r
                        lhsT = kv4[:, g, :, bc:bc + HB + 1:HB]
                        rhs = qv4[:, g, :, bc:bc + HB + 1:HB]
                        nc.tensor.matmul(
                            lg[32 * q:32 * q + 2 * A,
                               2 * A * r:2 * A * r + 2 * A],
                            lhsT, rhs, start=True, stop=True,
                            tile_position=(0, 32 * q))
                mlog = wpool.tile([128, LGF], BF16, tag="mlog",
                                  bufs=MLOG_BUFS)
                lg3 = lg.rearrange("p (s i) -> p s i", i=2 * A)
                ml3 = mlog[:, :].rearrange("p (s i) -> p s i", i=2 * A)
                nc.vector.copy_predicated(ml3, mk, lg3)
                expm = wpool.tile([128, LGF], BF16, tag="expm", bufs=9)
                nc.scalar.activation(
                    expm[:, :], mlog[:, :], mybir.ActivationFunctionType.Exp,
                    scale=0.125)
                e_tiles.append(expm)

            # ---------- stage D2: denominators + P = expm/den ------------
            # software-pipelined: den(b+1) covers the recip(b) latency
            rden_tiles = [None] * (2 * G)
            p_tiles = [None] * (2 * G)

            def emit_den(beta):
                den = ppool.tile([128, 512], FP32, tag="big", bufs=8)
                nc.tensor.matmul(den[:8, :LGF], c_dones[:, :],
                                 e_tiles[beta][:, :], start=True, stop=True)
                dsafe = wpool.tile([8, LGF], FP32, tag="dsafe", bufs=3)
                nc.vector.tensor_scalar_max(dsafe[:, :], den[:8, :LGF], 1e-6)
                rden = wpool.tile([8, LGF], FP32, tag="rden", bufs=3)
                nc.vector.reciprocal_approx_fast(rden[:, :], dsafe[:, :])
                rden_tiles[beta] = rden

            def emit_rrep(beta):
                rrep = ppool.tile([128, 512], FP32, tag="big", bufs=8)
                nc.tensor.matmul(rrep[:, :LGF], c_exp[:, :],
                                 rden_tiles[beta][:, :], start=True, stop=True)
                ptile = wpool.tile([128, LGF], BF16, tag="ptile", bufs=9)
                nc.vector.tensor_tensor(ptile[:, :], e_tiles[beta][:, :],
                                        rrep[:, :LGF], AluOpType.mult)
                p_tiles[beta] = ptile

            emit_den(0)
            for beta in range(1, 2 * G):
                emit_den(beta)
                emit_rrep(beta - 1)
            emit_rrep(2 * G - 1)

            # ---------- stage E: paired attention + output ----------
            for g in range(4):
                osb = wpool.tile([128, 4 * 8 * D], BF16, tag="osb", bufs=2)
                for m in range(4):
                    at = ppool.tile([128, 512], FP32, tag="big", bufs=8)
                    for q in range(4):
                        for sl in range(8):
                            bc = 32 * q + 8 * m + sl
                            s = 8 * (m % 2) + sl
                            t, kk = (8 * m + sl) // 2, sl % 2
                            pt = p_tiles[g * 2 + m // 2]
                            lhsT = pt[32 * q:32 * q + 2 * A,
                                      2 * A * s:2 * A * s + 2 * A]
                            rhs = v_tiles[t][32 * q:32 * q + 2 * A,
                                             kk * 256 + g * D:
                                             kk * 256 + (g + 1) * D]
                            nc.tensor.matmul(
                                at[32 * q:32 * q + 2 * A,
                                   sl * D:(sl + 1) * D],
                                lhsT, rhs, start=True, stop=True,
                                tile_position=(32 * q, 32 * q))
                    dst = osb[:, m * 8 * D:(m + 1) * 8 * D]
                    nc.vector.tensor_copy(dst, at[:, :])
                # b = 256c + 128bp + 32q + 8m + sl; osb row = 32q + 2a + bp
                ov = out[g, :, :, :].rearrange(
                    "a (c2 bp q m sl) d -> c2 q a bp m (sl d)",
                    c2=NCHUNK, bp=2, q=4, m=4)
                for q in range(4):
                    for bp in range(2):
                        nc.sync.dma_start(
                            out=ov[c, q, :, bp],
                            in_=osb[32 * q + bp:32 * q + bp + 2 * A:2, :]
                            .rearrange("a (m f) -> a m f", m=4))
    return nc


# ----------------------------------------------------------------------------
# host wrapper
# ----------------------------------------------------------------------------
_CACHE = {}


def _prep_consts(W_enc, b_enc, W_s, b_s, W_k, W_q, W_v, b_v):
    bf = lambda x: np.ascontiguousarray(x.astype(bfloat16))
    f32 = lambda x: np.ascontiguousarray(x.astype(np.float32))
    we = np.transpose(W_enc, (1, 0, 2))              # [160, A, H]
    wenc1 = bf(we[:S].reshape(S, A * H))
    wenc2 = bf(we[S:].reshape(ACT, A * H))
    ws_p = bf(np.transpose(W_s, (1, 0, 2)).reshape(S, A * H))

    # [h, (p, gpair, hh, g', d)]: W[p][2gp+g'][hh*128+h, d]
    wqk = np.zeros((128, 2, 2, HH, 2, D), np.float32)
    for p, W in enumerate((W_q, W_k)):
        for gp in range(2):
            for hh in range(HH):
                for gq in range(2):
                    wqk[:, p, gp, hh, gq, :] = W[2 * gp + gq,
                                                 hh * 128:(hh + 1) * 128, :]
    wqk2 = bf(wqk.reshape(128, 2 * 2 * HH * 128))

    wv = np.zeros((128, HH, G * D), np.float32)
    for hh in range(HH):
        wv[:, hh, :] = np.transpose(
            W_v[:, hh * 128:(hh + 1) * 128, :], (1, 0, 2)).reshape(128, G * D)
    wv_p = bf(wv.reshape(128, HH * G * D))
    benc_p = f32(np.transpose(b_enc.reshape(A, HH, 128), (2, 1, 0)).reshape(128, HH * A))
    bs_p = f32(np.transpose(b_s.reshape(A, HH, 128), (2, 1, 0)).reshape(128, HH * A))
    bvrow = bf(b_v.reshape(1, G * D))
    onesrow = bf(np.ones((1, 128), np.float32))
    mk = np.zeros((128, 2 * A), np.uint8)
    for cs in range(4):
        for bp in range(2):
            for jj in range(A):
                for i in range(A):
                    if jj != i:
                        mk[32 * cs + 2 * jj + bp, 2 * i + bp] = 1
    maskkeep = np.ascontiguousarray(mk)
    dn = np.zeros((128, 8), np.float32)
    for cs in range(4):
        for bp in range(2):
            for jj in range(A):
                dn[32 * cs + 2 * jj + bp, 2 * cs + bp] = 1.0
    denones = bf(dn)
    expand = f32(dn.T.copy())                        # [8, 128]
    return dict(wenc1=wenc1, wenc2=wenc2, ws_p=ws_p, wqk2=wqk2,
                wv_p=wv_p, benc_p=benc_p, bs_p=bs_p, bvrow=bvrow,
                onesrow=onesrow, maskkeep=maskkeep, denones=denones,
                expand=expand)


def kernel(states, actions, W_enc, b_enc, W_s, b_s, W_k, W_q, W_v, b_v):
    states = np.asarray(states, np.float32)
    actions = np.asarray(actions, np.float32)
    consts = _prep_consts(np.asarray(W_enc, np.float32), np.asarray(b_enc, np.float32),
                          np.asarray(W_s, np.float32), np.asarray(b_s, np.float32),
                          np.asarray(W_k, np.float32), np.asarray(W_q, np.float32),
                          np.asarray(W_v, np.float32), np.asarray(b_v, np.float32))
    key = (bool(np.any(b_enc)), bool(np.any(b_s)), bool(np.any(b_v)))
    if key not in _CACHE:
        nc = bacc.Bacc("TRN2", target_bir_lowering=False, debug=False,
                       num_devices=N_CORES)
        _build(nc, *key)
        nc.compile()
        _CACHE[key] = nc
    nc = _CACHE[key]

    in_maps = []
    for i in range(N_CORES):
        sl = slice(i * B_LOC, (i + 1) * B_LOC)
        stT = np.ascontiguousarray(
            np.transpose(states[:, sl, :], (2, 0, 1)).reshape(S, A * B_LOC).astype(bfloat16))
        acT = np.ascontiguousarray(
            np.transpose(actions[:, sl, :], (2, 0, 1)).reshape(ACT, A * B_LOC).astype(bfloat16))
        m = dict(statesT_p=stT, actionsT_p=acT)
        m.update(consts)
        in_maps.append(m)

    global _last_in_maps
    _last_in_maps = in_maps
    res = run_bass_kernel_spmd(nc, in_maps, core_ids=list(range(N_CORES)))
    outs = [np.asarray(res.results[i]["out"], dtype=np.float32)
            for i in range(N_CORES)]
    return np.concatenate(outs, axis=2)


if __name__ == "__main__":
    nc = bacc.Bacc("TRN2", target_bir_lowering=False, debug=False,
                   num_devices=N_CORES)
    _build(nc, False, False, False)
    nc.compile()
    print("build ok")
